# revision 1
# baseline (speedup 1.0000x reference)
"""Trainium2 Bass kernel: HMM forward algorithm (log-space) for AugmentedModel.log_prob.

Reformulation: run the forward recurrence in probability domain with periodic
rescaling.  Per step t (129 steps):
    w   = alpha ⊙ q_t              (q_t = exp(emission+policy log-prob row))
    u   = w @ P[a_t[b]]            (per-batch transition matrix, 8 choices)
    alpha ∝ u                      (normalize every NORM_EVERY steps, lagged)
log_prob[b] = sum of logs of the removed scales (telescopes exactly).

Per-batch transition selection is done by zero-masking the matmul stationary
operand per action and accumulating all 8 actions into PSUM (masks are
disjoint one-hots over actions, so the accumulated sum IS the selection).
The 8 actions are packed 2-per-column-group via tile_position so 4 matmul
streams run concurrently in the PE array.

Sharding: data-parallel over batch B=128 -> 16 episodes per core, tables
replicated; no collectives (each core's recurrence is independent).

Emission rows lq_orda[t,b,:] are computed on device as a one-hot matmul
(one-hot over the concatenated [obs(64)|rew(16)|done(2)|action(8)] vocab,
built host-side from the int index tensors) against the stacked log tables,
then exponentiated on the Scalar engine.
"""

import numpy as np
from contextlib import ExitStack

T, B, S, A, NO, NR = 128, 128, 512, 8, 64, 16
TT = T + 1
NCORES = 8
BC = B // NCORES          # 16 episodes per core
KC = 4                    # 512 states = 4 chunks of 128 partitions
NORM_EVERY = 4
NCOL = 6                  # one-hot matmul N-chunks: 2064 = 6*344
NCHUNK = (TT * BC) // NCOL
VOC = NO + NR + 2 + A     # 90
COLTILE = True


def _host_prep(regime, seq_o, seq_r, seq_d, seq_a):
    """Index-only preprocessing: one-hot encodings + action masks (no table math)."""
    d_all = np.concatenate([seq_d, np.ones((1, B), np.int32)], 0)        # [TT,B]
    d_cum = np.maximum.accumulate(d_all, 0)                              # [TT,B]
    was_d = np.concatenate([np.zeros((1, B), np.int32), d_cum[:-1]], 0)  # [TT,B]
    a_full = np.concatenate([seq_a, np.zeros((1, B), np.int32)], 0)      # [TT,B]

    oh = np.zeros((TT, B, VOC), np.float32)
    tt, bb = np.meshgrid(np.arange(TT), np.arange(B), indexing="ij")
    live = was_d == 0
    oh[tt[live], bb[live], seq_o[live]] = 1.0
    oh[tt[live], bb[live], NO + seq_r[live]] = 1.0
    oh[tt[live], bb[live], NO + NR + d_cum[live]] = 1.0
    act = (d_cum == 0) & (regime[None, :] == 0)
    oh[tt[act], bb[act], NO + NR + 2 + a_full[act]] = 1.0

    msk = (a_full[:, None, :] == np.arange(A)[None, :, None]).astype(np.float32)  # [TT,A,B]
    return oh, msk, a_full


def _bc_insert(ap, axis, count):
    """Insert a 0-stride (broadcast) dim of size `count` at position `axis`."""
    import concourse.bass as bass
    lst = [list(d) for d in ap.ap]
    lst.insert(axis, [0, count])
    return bass.AP(ap.tensor, ap.offset, lst)


def _build_nc(nsteps=TT, nreps=1):
    import concourse.bass as bass  # noqa: F401
    import concourse.bacc as bacc
    import concourse.mybir as mybir
    import concourse.tile as tile

    f32 = mybir.dt.float32
    f32r = mybir.dt.float32r
    bf16 = mybir.dt.bfloat16
    i32 = mybir.dt.int32
    EXP = mybir.ActivationFunctionType.Exp
    LN = mybir.ActivationFunctionType.Ln
    CPY = mybir.ActivationFunctionType.Copy
    MUL = mybir.AluOpType.mult
    ADD = mybir.AluOpType.add
    SHR = mybir.AluOpType.logical_shift_right
    BAND = mybir.AluOpType.bitwise_and
    BOR = mybir.AluOpType.bitwise_or
    AX = mybir.AxisListType.X

    nc = bacc.Bacc(None, target_bir_lowering=False)

    oh_d = nc.dram_tensor("oh", [VOC, TT * BC], f32r, kind="ExternalInput")
    tbl_d = nc.dram_tensor("tbl", [VOC, S], f32r, kind="ExternalInput")
    pt_d = nc.dram_tensor("ptab", [128, KC, A, S], f32, kind="ExternalInput")
    li_d = nc.dram_tensor("linit", [128, KC], f32, kind="ExternalInput")
    mk_d = nc.dram_tensor("msk", [TT, 128, A, BC], f32, kind="ExternalInput")
    id_d = nc.dram_tensor("ident", [BC, BC], f32, kind="ExternalInput")
    out_d = nc.dram_tensor("out", [BC, 1], f32, kind="ExternalOutput")

    with tile.TileContext(nc) as tc, ExitStack() as ctx:
        const = ctx.enter_context(tc.tile_pool(name="const", bufs=1))
        mpool = ctx.enter_context(tc.tile_pool(name="mask", bufs=4))
        wpool = ctx.enter_context(tc.tile_pool(name="w", bufs=2))
        w8pool = ctx.enter_context(tc.tile_pool(name="w8", bufs=2))
        spool = ctx.enter_context(tc.tile_pool(name="ssum", bufs=2))
        nrm = ctx.enter_context(tc.tile_pool(name="nrm", bufs=2))
        pp = ctx.enter_context(tc.tile_pool(name="ppsum", bufs=2, space="PSUM"))
        ptp = ctx.enter_context(tc.tile_pool(name="tpsum", bufs=2, space="PSUM"))

        ptab = const.tile([128, KC, A, S], bf16)
        qbuf = const.tile([128, KC, TT * BC], f32)
        tbl = const.tile([VOC, S], f32r)
        oh = const.tile([VOC, TT * BC], f32r)
        ident = const.tile([BC, BC], f32)
        alpha0 = const.tile([128, KC, 1], f32)
        li = const.tile([128, KC], f32)
        logacc = const.tile([BC, 1], f32)

        nc.sync.dma_start(tbl[:], tbl_d[:])
        nc.sync.dma_start(oh[:], oh_d[:])
        nc.sync.dma_start(ident[:], id_d[:])
        nc.sync.dma_start(li[:], li_d[:])
        nc.scalar.activation(alpha0[:, :, 0], li[:], EXP)
        nc.vector.memset(logacc[:], 0.0)

        # transition tables: DMA f32 chunk, exp -> bf16
        for kc in range(KC):
            for a in range(A):
                stg = mpool.tile([128, S], f32, tag="pstg")
                nc.sync.dma_start(stg[:], pt_d[:, kc, a, :])
                nc.scalar.activation(ptab[:, kc, a, :], stg[:], EXP)

        # emission rows: lq = tbl.T @ onehot, then q = exp(lq)
        for mc in range(KC):
            for j in range(NCOL):
                lq = pp.tile([128, NCHUNK], f32, tag="lq")
                nc.tensor.matmul(
                    lq[:],
                    tbl[:, mc * 128:(mc + 1) * 128],
                    oh[:, j * NCHUNK:(j + 1) * NCHUNK],
                    start=True, stop=True,
                )
                nc.scalar.activation(
                    qbuf[:, mc, j * NCHUNK:(j + 1) * NCHUNK], lq[:], EXP
                )

        for _rep in range(nreps):
         uT_prev = None
         recip = None
         for t in range(nsteps):
            m = mpool.tile([128, A, BC], f32, tag="m")
            nc.sync.dma_start(m[:], mk_d[t])

            # w = alpha ⊙ q_t   (T-layout [s_part, kc, b])
            w = wpool.tile([128, KC, BC], f32, tag="w")
            qs = qbuf[:, :, t * BC:(t + 1) * BC]
            if t == 0:
                nc.vector.tensor_tensor(
                    w[:], qs, alpha0[:].broadcast_to((128, KC, BC)), MUL
                )
            else:
                nc.vector.tensor_tensor(w[:], qs, uT_prev[:], MUL)

            # w8[p,a,kc,b] = w[p,kc,b] * mask[p,a,b]
            w8 = w8pool.tile([128, A, KC, BC], bf16, tag="w8")
            nc.vector.tensor_tensor(
                w8[:],
                _bc_insert(w[:], 1, A),
                _bc_insert(m[:], 2, KC),
                MUL,
            )

            # u = sum_a (w8[a] @ P[a]) : masked accumulate; 4 col-group strips
            strips = pp.tile([128, S], f32, tag="strips")
            for a in range(A):
                j = (a % 4) if COLTILE else 0
                row0 = 32 * j
                first = (a < 4) if COLTILE else (a == 0)
                last = (a >= 4) if COLTILE else (a == A - 1)
                for kc in range(KC):
                    nc.tensor.matmul(
                        strips[row0:row0 + BC, :],
                        w8[:, a, kc, :],
                        ptab[:, kc, a, :],
                        start=(first and kc == 0),
                        stop=(last and kc == KC - 1),
                        tile_position=(0, 32 * j) if COLTILE else None,
                    )

            # strips -> SBUF copies (B-layout), split DVE/ACT, with free
            # accum_out row-sums and the lagged 1/ell rescale folded in.
            is_apply = t > NORM_EVERY and (t - 2) % NORM_EVERY == 0
            is_event = t % NORM_EVERY == 0 and t > 0
            cs = [spool.tile([BC, S], f32, tag=f"c{j}", name=f"c{j}")
                  for j in range(4)]
            es = [nrm.tile([BC, 1], f32, tag=f"e{j}", name=f"e{j}")
                  for j in range(4)]
            for j, c in enumerate(cs):
                sp = strips[32 * j:32 * j + BC, :]
                ao = es[j][:] if is_event else None
                if j < 2:
                    sc = recip[:] if is_apply else 1.0
                    if ao is not None:
                        nc.vector.tensor_scalar(c[:], sp, sc, 0.0, MUL, ADD,
                                                accum_out=ao)
                    else:
                        nc.vector.tensor_scalar(c[:], sp, sc, None, MUL)
                else:
                    nc.scalar.activation(c[:], sp, CPY,
                                         scale=recip[:] if is_apply else 1.0,
                                         accum_out=ao)

            # normalization event: measure mass, log it
            if is_event:
                ell = nrm.tile([BC, 1], f32, tag="ell")
                nc.vector.tensor_tensor(ell[:], es[0][:], es[1][:], ADD)
                nc.vector.tensor_tensor(ell[:], ell[:], es[2][:], ADD)
                nc.vector.tensor_tensor(ell[:], ell[:], es[3][:], ADD)
                # ln(ell) via frexp: ACT Ln is only trustworthy near 1, and
                # ell is ~1e-17 here. ln(m*2^e) = Ln(m) + (e-127)*ln2.
                e_t = nrm.tile([BC, 1], i32, tag="e_t")
                nc.vector.tensor_scalar(e_t[:], ell[:].bitcast(i32), 23, None, SHR)
                ef = nrm.tile([BC, 1], f32, tag="ef")
                nc.vector.tensor_copy(ef[:], e_t[:])
                m_t = nrm.tile([BC, 1], i32, tag="m_t")
                nc.vector.tensor_scalar(m_t[:], ell[:].bitcast(i32),
                                        0x007FFFFF, 0x3F800000, BAND, BOR)
                lnb = nrm.tile([BC, 1], f32, tag="lnb")
                nc.scalar.activation(lnb[:], m_t[:].bitcast(f32), LN)
                esc = nrm.tile([BC, 1], f32, tag="esc")
                nc.scalar.activation(esc[:], ef[:], CPY,
                                     bias=-88.02969193111305,
                                     scale=0.6931471805599453)
                nc.vector.tensor_tensor(logacc[:], logacc[:], lnb[:], ADD)
                nc.vector.tensor_tensor(logacc[:], logacc[:], esc[:], ADD)
                if t < TT - 1:
                    rec = nrm.tile([BC, 1], f32, tag="rec")
                    nc.vector.reciprocal(rec[:], ell[:])
                    recip = rec

            # transpose u back to T-layout, accumulating the 4 strip copies
            if t < nsteps - 1:
                uT = ptp.tile([128, KC, BC], f32, tag="uT")
                for kc in range(KC):
                    for j, c in enumerate(cs):
                        nc.tensor.matmul(
                            uT[:, kc, :], c[:, kc * 128:(kc + 1) * 128],
                            ident[:], is_transpose=True,
                            start=(j == 0), stop=(j == 3),
                        )
                uT_prev = uT

        nc.sync.dma_start(out_d[:], logacc[:])

    nc.compile()
    return nc


_NC = None


def _get_nc():
    global _NC
    if _NC is None:
        _NC = _build_nc()
    return _NC


def make_in_maps(regime, seq_o, seq_r, seq_d, seq_a,
                 log_init, log_trans, log_emit_o, log_emit_r, log_emit_d,
                 log_policy):
    oh, msk, _ = _host_prep(
        np.asarray(regime), np.asarray(seq_o), np.asarray(seq_r),
        np.asarray(seq_d), np.asarray(seq_a),
    )
    tbl = np.concatenate(
        [log_emit_o, log_emit_r, log_emit_d, log_policy], 0
    ).astype(np.float32)                                         # [90, 512]
    ptab = np.ascontiguousarray(
        np.asarray(log_trans, np.float32).reshape(A, KC, 128, S).transpose(2, 1, 0, 3)
    )                                                            # [128,KC,A,S]
    linit = np.ascontiguousarray(np.asarray(log_init, np.float32).reshape(KC, 128).T)
    ident = np.eye(BC, dtype=np.float32)

    in_maps = []
    for c in range(NCORES):
        bs = c * BC
        ohc = np.ascontiguousarray(
            oh[:, bs:bs + BC, :].transpose(2, 0, 1).reshape(VOC, TT * BC)
        )
        mskc = np.ascontiguousarray(
            np.broadcast_to(msk[:, None, :, bs:bs + BC], (TT, 128, A, BC))
        )
        in_maps.append({
            "oh": ohc, "tbl": tbl, "ptab": ptab, "linit": linit,
            "msk": mskc, "ident": ident,
        })
    return in_maps


def kernel(regime, seq_o, seq_r, seq_d, seq_a,
           log_init, log_trans, log_emit_o, log_emit_r, log_emit_d,
           log_policy, _trace=False):
    from concourse.bass_utils import run_bass_kernel_spmd

    nc = _get_nc()
    in_maps = make_in_maps(
        regime, seq_o, seq_r, seq_d, seq_a, log_init, log_trans,
        log_emit_o, log_emit_r, log_emit_d, log_policy,
    )
    res = run_bass_kernel_spmd(nc, in_maps, core_ids=list(range(NCORES)),
                               trace=_trace)
    out = np.concatenate([r["out"].reshape(BC) for r in res.results])
    if _trace:
        kernel._last_results = res
    return out.astype(np.float32)



# revision 2
# speedup vs baseline: 3.2705x; 3.2705x over previous
"""Trainium2 Bass kernel: HMM forward algorithm (log-space) for AugmentedModel.log_prob.

Probability-domain recurrence with fp8 DoubleRow matmuls:
    w   = alpha ⊙ q_t                  (q_t pre-scaled per (t,b) so mass ~ 1)
    w8  = fp8_e5m2(w ⊙ mask_a)         (per-action one-hot masks, disjoint)
    u   = Σ_a w8_a @ P8[a]             (P8 = fp8_e4m3(512·exp(log_trans)))
    s_t = Σ_s u; alpha ∝ u / s_{t-1}   (lag-1 rescale keeps mass ≈ 512·e^δ)
log_prob[b] = Σ_t ln s_t + Σ_t C_tb − 129·ln 512   (host-side; C = per-(t,b)
pre-scale constants injected via an extra one-hot vocab row).

Matmuls use MatmulPerfMode.DoubleRow: two 128-row K-tiles per instruction at
0.5 cycles/row — 16 matmuls/step instead of 32 bf16 ones at 1 cycle/row, a 4x
cut in PE streaming time.  All 8 action matmuls accumulate into one PSUM tile
(masks are disjoint one-hots, so the sum IS the per-batch selection).

Sharding: data-parallel over batch B=128 -> 16 episodes per core, tables
replicated; no collectives (each core's recurrence is independent).
"""

import numpy as np
from contextlib import ExitStack

T, B, S, A, NO, NR = 128, 128, 512, 8, 64, 16
TT = T + 1
NCORES = 8
BC = B // NCORES          # 16 episodes per core
KC = 4                    # 512 states = 4 chunks of 128 partitions
NCOL = 6                  # one-hot matmul N-chunks: 2064 = 6*344
NCHUNK = (TT * BC) // NCOL
VOC = NO + NR + 2 + A     # 90 one-hot rows
VOCC = VOC + 1            # +1 row carrying the per-(t,b) scale constant
LN512 = float(np.log(512.0))


def _host_prep(regime, seq_o, seq_r, seq_d, seq_a,
               log_emit_o, log_emit_r, log_emit_d, log_policy):
    """Index preprocessing: one-hots, action masks, and per-(t,b) scale C."""
    d_all = np.concatenate([seq_d, np.ones((1, B), np.int32)], 0)        # [TT,B]
    d_cum = np.maximum.accumulate(d_all, 0)                              # [TT,B]
    was_d = np.concatenate([np.zeros((1, B), np.int32), d_cum[:-1]], 0)  # [TT,B]
    a_full = np.concatenate([seq_a, np.zeros((1, B), np.int32)], 0)      # [TT,B]

    # emission+policy log rows (host, f32) -> per-(t,b) scale constants
    lq = (log_emit_o[seq_o] + log_emit_r[seq_r]
          + log_emit_d[d_cum]).astype(np.float32)                        # [TT,B,S]
    lq[was_d == 1] = 0.0
    lq_a = log_policy[a_full].astype(np.float32)
    lq_a[d_cum == 1] = 0.0
    lq_a[:, regime == 1, :] = 0.0
    lq += lq_a
    mx = lq.max(axis=2)
    C = (mx + np.log(np.exp(lq - mx[:, :, None]).mean(axis=2))).astype(np.float32)

    oh = np.zeros((TT, B, VOCC), np.float32)
    tt, bb = np.meshgrid(np.arange(TT), np.arange(B), indexing="ij")
    live = was_d == 0
    oh[tt[live], bb[live], seq_o[live]] = 1.0
    oh[tt[live], bb[live], NO + seq_r[live]] = 1.0
    oh[tt[live], bb[live], NO + NR + d_cum[live]] = 1.0
    act = (d_cum == 0) & (regime[None, :] == 0)
    oh[tt[act], bb[act], NO + NR + 2 + a_full[act]] = 1.0
    oh[:, :, VOC] = -C                                                   # scale row

    msk = (a_full[:, None, :] == np.arange(A)[None, :, None]).astype(np.float32)
    return oh, msk, C


def _bc_insert(ap, axis, count):
    """Insert a 0-stride (broadcast) dim of size `count` at position `axis`."""
    import concourse.bass as bass
    lst = [list(d) for d in ap.ap]
    lst.insert(axis, [0, count])
    return bass.AP(ap.tensor, ap.offset, lst)


def _build_nc(nsteps=TT, nreps=1):
    import concourse.bass as bass  # noqa: F401
    import concourse.bacc as bacc
    import concourse.mybir as mybir
    import concourse.tile as tile

    f32 = mybir.dt.float32
    f32r = mybir.dt.float32r
    bf16 = mybir.dt.bfloat16
    e4 = mybir.dt.float8e4
    e5 = mybir.dt.float8e5
    EXP = mybir.ActivationFunctionType.Exp
    CPY = mybir.ActivationFunctionType.Copy
    MUL = mybir.AluOpType.mult
    DR = mybir.MatmulPerfMode.DoubleRow

    nc = bacc.Bacc(None, target_bir_lowering=False)

    oh_d = nc.dram_tensor("oh", [VOCC, TT * BC], f32r, kind="ExternalInput")
    tbl_d = nc.dram_tensor("tbl", [VOCC, S], f32r, kind="ExternalInput")
    pt_d = nc.dram_tensor("ptab", [128, A, KC, S], e4, kind="ExternalInput")
    li_d = nc.dram_tensor("linit", [128, KC], f32, kind="ExternalInput")
    mk_d = nc.dram_tensor("msk", [TT, 128, A, BC], bf16, kind="ExternalInput")
    id_d = nc.dram_tensor("ident", [BC, BC], bf16, kind="ExternalInput")
    out_d = nc.dram_tensor("out", [BC, TT], f32, kind="ExternalOutput")

    with tile.TileContext(nc) as tc, ExitStack() as ctx:
        const = ctx.enter_context(tc.tile_pool(name="const", bufs=1))
        mpool = ctx.enter_context(tc.tile_pool(name="mask", bufs=4))
        wpool = ctx.enter_context(tc.tile_pool(name="w", bufs=2))
        w8pool = ctx.enter_context(tc.tile_pool(name="w8", bufs=2))
        spool = ctx.enter_context(tc.tile_pool(name="ssum", bufs=2))
        nrm = ctx.enter_context(tc.tile_pool(name="nrm", bufs=3))
        pp = ctx.enter_context(tc.tile_pool(name="ppsum", bufs=2, space="PSUM"))
        ptp = ctx.enter_context(tc.tile_pool(name="tpsum", bufs=2, space="PSUM"))
        lqp = ctx.enter_context(tc.tile_pool(name="lqpsum", bufs=2, space="PSUM"))

        ptab = const.tile([128, A, KC, S], e4)
        qbuf = const.tile([128, KC, TT * BC], bf16)
        tbl = const.tile([VOCC, S], f32r)
        oh = const.tile([VOCC, TT * BC], f32r)
        ident = const.tile([BC, BC], bf16)
        alpha0 = const.tile([128, KC, 1], f32)
        li = const.tile([128, KC], f32)
        ellbuf = const.tile([BC, TT], f32)

        nc.sync.dma_start(tbl[:], tbl_d[:])
        nc.sync.dma_start(oh[:], oh_d[:])
        nc.sync.dma_start(ident[:], id_d[:])
        nc.sync.dma_start(li[:], li_d[:])
        nc.sync.dma_start(ptab[:], pt_d[:])
        nc.scalar.activation(alpha0[:, :, 0], li[:], EXP)

        # emission rows: lq = tbl.T @ onehot (+C row), then q = exp(lq) -> bf16
        for mc in range(KC):
            for j in range(NCOL):
                lq = lqp.tile([128, NCHUNK], f32, tag="lq")
                nc.tensor.matmul(
                    lq[:],
                    tbl[:, mc * 128:(mc + 1) * 128],
                    oh[:, j * NCHUNK:(j + 1) * NCHUNK],
                    start=True, stop=True,
                )
                nc.scalar.activation(
                    qbuf[:, mc, j * NCHUNK:(j + 1) * NCHUNK], lq[:], EXP
                )

        for _rep in range(nreps):
         recip = None
         for t in range(nsteps):
            m = mpool.tile([128, A, BC], bf16, tag="m")
            nc.sync.dma_start(m[:], mk_d[t])

            # w = alpha ⊙ q_t   (T-layout [s_part, kc, b])
            w = wpool.tile([128, KC, BC], bf16, tag="w")
            qs = qbuf[:, :, t * BC:(t + 1) * BC]
            if t == 0 and _rep == 0:
                nc.vector.tensor_tensor(
                    w[:], qs, alpha0[:].broadcast_to((128, KC, BC)), MUL
                )
            else:
                nc.vector.tensor_tensor(w[:], qs, uT_prev[:], MUL)

            # w8[p,a,kc,b] = fp8e5(w[p,kc,b] * mask[p,a,b])
            w8 = w8pool.tile([128, A, KC, BC], e5, tag="w8")
            nc.vector.tensor_tensor(
                w8[:],
                _bc_insert(w[:], 1, A),
                _bc_insert(m[:], 2, KC),
                MUL,
            )

            # u = Σ_a w8_a @ P8[a] : 16 DoubleRow matmuls into one PSUM tile
            u = pp.tile([BC, S], f32, tag="u")
            for j in range(2):
                for a in range(A):
                    nc.tensor.matmul(
                        u[:],
                        w8[:, a, 2 * j:2 * j + 2, :],
                        ptab[:, a, 2 * j:2 * j + 2, :],
                        start=(j == 0 and a == 0),
                        stop=(j == 1 and a == A - 1),
                        perf_mode=DR,
                    )

            # c = u * recip_{t-1} (bf16), mass accum -> ellbuf[:, t]
            c = spool.tile([BC, S], bf16, tag="c")
            nc.scalar.activation(c[:], u[:], CPY,
                                 scale=recip[:] if recip is not None else 1.0,
                                 accum_out=ellbuf[:, t:t + 1])
            rec = nrm.tile([BC, 1], f32, tag="rec")
            nc.vector.reciprocal(rec[:], ellbuf[:, t:t + 1])
            recip = rec

            # transpose u back to T-layout for the next step
            if t < nsteps - 1 or _rep < nreps - 1:
                uT = ptp.tile([128, KC, BC], bf16, tag="uT")
                for kc in range(KC):
                    nc.tensor.matmul(
                        uT[:, kc, :], c[:, kc * 128:(kc + 1) * 128],
                        ident[:], is_transpose=True,
                        start=True, stop=True,
                    )
                uT_prev = uT

        nc.sync.dma_start(out_d[:], ellbuf[:])

    nc.compile()
    return nc


_NC = None


def _get_nc():
    global _NC
    if _NC is None:
        _NC = _build_nc()
    return _NC


def make_in_maps(regime, seq_o, seq_r, seq_d, seq_a,
                 log_init, log_trans, log_emit_o, log_emit_r, log_emit_d,
                 log_policy):
    import ml_dtypes

    oh, msk, C = _host_prep(
        np.asarray(regime), np.asarray(seq_o), np.asarray(seq_r),
        np.asarray(seq_d), np.asarray(seq_a),
        np.asarray(log_emit_o, np.float32), np.asarray(log_emit_r, np.float32),
        np.asarray(log_emit_d, np.float32), np.asarray(log_policy, np.float32),
    )
    tbl = np.concatenate(
        [log_emit_o, log_emit_r, log_emit_d, log_policy,
         np.ones((1, S), np.float32)], 0
    ).astype(np.float32)                                         # [91, 512]
    P8 = (512.0 * np.exp(np.asarray(log_trans, np.float64))).astype(np.float32)
    ptab = np.ascontiguousarray(
        P8.reshape(A, KC, 128, S).transpose(2, 0, 1, 3)
    ).astype(ml_dtypes.float8_e4m3)                              # [128,A,KC,S]
    linit = np.ascontiguousarray(np.asarray(log_init, np.float32).reshape(KC, 128).T)
    ident = np.eye(BC, dtype=ml_dtypes.bfloat16)

    in_maps = []
    for c in range(NCORES):
        bs = c * BC
        ohc = np.ascontiguousarray(
            oh[:, bs:bs + BC, :].transpose(2, 0, 1).reshape(VOCC, TT * BC)
        )
        mskc = np.ascontiguousarray(
            np.broadcast_to(msk[:, None, :, bs:bs + BC], (TT, 128, A, BC))
        ).astype(ml_dtypes.bfloat16)
        in_maps.append({
            "oh": ohc, "tbl": tbl, "ptab": ptab, "linit": linit,
            "msk": mskc, "ident": ident,
        })
    return in_maps, C


def kernel(regime, seq_o, seq_r, seq_d, seq_a,
           log_init, log_trans, log_emit_o, log_emit_r, log_emit_d,
           log_policy, _trace=False):
    from concourse.bass_utils import run_bass_kernel_spmd

    nc = _get_nc()
    in_maps, C = make_in_maps(
        regime, seq_o, seq_r, seq_d, seq_a, log_init, log_trans,
        log_emit_o, log_emit_r, log_emit_d, log_policy,
    )
    res = run_bass_kernel_spmd(nc, in_maps, core_ids=list(range(NCORES)),
                               trace=_trace)
    ell = np.concatenate([r["out"].reshape(BC, TT) for r in res.results])  # [B,TT]
    logp = (np.log(ell.astype(np.float64)).sum(1)
            + C.astype(np.float64).sum(0) - TT * LN512)
    if _trace:
        kernel._last_results = res
    return logp.astype(np.float32)
